# revision 12
# baseline (speedup 1.0000x reference)
"""ExemplarAttention Trainium2 kernel (8 NeuronCores, exemplar-sharded).

logits[b,c] = gamma * log(sum_{n:label[n]=c} exp(-beta * sum_k w_k (x[b,k]-e[n,k])^2) + eps)

Strategy (v2 — exemplar/N-sharded, transposed GEMM):
  - Shard the N=16384 exemplars across the 8 cores (~2048 each) and
    replicate the batch. Per-core DMA drops from 8MB (replicated bank)
    to ~1.6MB, and the per-class scatter-add becomes a host-side gather.
  - Transposed GEMM orientation: psum[n_part, b_free] = S * cross with
    exemplars on psum PARTITIONS (stationary = exemplar features, moving
    = S*x*w, both fp8 DoubleRow, K=2x256). The per-exemplar -beta*e2w
    term is now a per-partition constant -> it rides the ScalarE
    activation BIAS, eliminating the baseline's whole aug-matmul pass.
  - Exemplars are packed so each partition holds NT=17 same-class
    exemplars (one "chunk"), sorted by e2w. Tile pairs then share one
    bias value (pair-mean of e2w; adjacent-in-sorted-order so the
    approximation error is ~1e-4 relative), letting ScalarE exp whole
    2-tile [128,2048] psum chunks in one instruction.
  - Per-class reduction = elementwise bf16 adds over the 17 tiles on the
    otherwise-idle VectorE (2x mode), into 3 accumulators that stream
    out early. Host: sum partitions by class, apply exp(-beta*x2w[b]),
    gamma*log(.+eps) in f64.
  - ScalarE's exp (~2.2M elements/core @ 1 elem/lane/cycle) is the
    irreducible bottleneck (~15-17us); PE (~14.5us@1.2GHz), DVE (~11us)
    and DMA (~5us) all hide under it.
"""

import os
from contextlib import ExitStack

import numpy as np

B, N, D, C = 1024, 16384, 512, 10
NCORES = 8
NT = 17                      # exemplar slots per partition (one class chunk)
NG = 2                       # DoubleRow K-groups (K=256 each)
NTILE = 512                  # matmul free dim (1 psum bank)
EPS = 1e-9
S_SCALE = 128.0              # fp8 scale applied to x*w
PAD_BIAS = -100.0            # bias for all-padding chunks: exp(-100) == 0

# psum chunks: singles at both ends (small first Act starts the pipeline
# early; the trailing singles' exps write straight to their own output
# buffer, taking VectorE off the tail), pairs between.
CHUNKS = ([(0,)] + [(2 * j + 1, 2 * j + 2) for j in range(7)]
          + [(15,), (16,)])
# acc groups: tiles 0-4 -> acc0, 5-10 -> acc1, 11-14 -> acc2 (VectorE
# accumulated); tiles 15, 16 -> acc3/acc4 written directly by ScalarE.
NACC = 5
ACT_DIRECT = {15: 3, 16: 4}          # tile -> direct output acc idx
ACC_LAST_CHUNK = {0: 2, 1: 5, 2: 7}  # acc idx -> chunk after which it's final


def _acc_of_tile(t):
    return 0 if t < 5 else (1 if t < 11 else 2)


N_WARMUP_MM = 7

_prog_cache = {}


def _np_dt(mybir, name):
    return mybir.dt.np(getattr(mybir.dt, name))


def _build_program(act_scale):
    import concourse.bass as bass  # noqa: F401
    import concourse.tile as tile
    from concourse import bacc, mybir

    fp8 = mybir.dt.float8e4
    bf16 = mybir.dt.bfloat16
    f32 = mybir.dt.float32
    DR = mybir.MatmulPerfMode.DoubleRow
    ADD = mybir.AluOpType.add

    nc = bacc.Bacc("TRN2", target_bir_lowering=False, debug=False,
                   num_devices=NCORES)

    # DRAM layouts mirror the SBUF layouts (partition-major) so each load
    # is a plain strided DMA.
    e_d = nc.dram_tensor("e_t", [128, NT, NG, 2, 128], fp8,
                         kind="ExternalInput").ap()
    # b-half-major xw layout: chunk0's h0 matmuls only need the first half
    # of the 512KB xw stream, halving the head-critical DMA.
    xw_d = nc.dram_tensor("xw_t", [128, 2, NG, 2, NTILE], fp8,
                          kind="ExternalInput").ap()
    bias_d = nc.dram_tensor("bias", [128, len(CHUNKS)], f32,
                            kind="ExternalInput").ap()
    out_d = nc.dram_tensor("acc", [NACC, 128, B], bf16,
                           kind="ExternalOutput").ap()

    with tile.TileContext(nc) as tc, ExitStack() as ctx:
        singles = ctx.enter_context(tc.tile_pool(name="singles", bufs=1))
        psum_pool = ctx.enter_context(tc.tile_pool(name="ps", bufs=2,
                                                   space="PSUM"))
        tmp_pool = ctx.enter_context(tc.tile_pool(name="tmp", bufs=3))

        xw_sb = singles.tile([128, 2, NG, 2, NTILE], fp8)
        e_sb = singles.tile([128, NT, NG, 2, 128], fp8)
        bias_sb = singles.tile([128, len(CHUNKS)], f32)
        accs = [singles.tile([128, B], bf16, name=f"acc{i}")
                for i in range(NACC)]

        # Warmup matmul operands: small memset tile, no DMA dependency.
        dmy = singles.tile([128, 2, 256], fp8)
        nc.vector.memset(dmy[:, :, :], 0.0)

        # Input DMA. Each dma_start costs ~620ns of issuing-sequencer time
        # and the transfers share ~330GB/s of queue bandwidth, so order by
        # need: chunk0 needs xw-h0, e-tile0 and bias first. Rings: sync
        # carries xw, gpsimd the exemplar tiles, the Act sequencer only
        # the tiny bias (done long before its first exp).
        nc.sync.dma_start(out=xw_sb[:, 0, :, :, :], in_=xw_d[:, 0, :, :, :])
        nc.gpsimd.dma_start(out=e_sb[:, 0:1, :, :, :],
                            in_=e_d[:, 0:1, :, :, :])
        nc.scalar.dma_start(out=bias_sb[:, :], in_=bias_d[:, :])
        nc.sync.dma_start(out=xw_sb[:, 1, :, :, :], in_=xw_d[:, 1, :, :, :])
        nc.gpsimd.dma_start(out=e_sb[:, 1:5, :, :, :],
                            in_=e_d[:, 1:5, :, :, :])
        nc.gpsimd.dma_start(out=e_sb[:, 5:11, :, :, :],
                            in_=e_d[:, 5:11, :, :, :])
        nc.gpsimd.dma_start(out=e_sb[:, 11:NT, :, :, :],
                            in_=e_d[:, 11:NT, :, :, :])

        # Warmup: ramp the PE clock + let the DMA stream get ahead.
        ps0 = psum_pool.tile([128, 2048], f32, tag="ps", name="ps0")
        for _ in range(N_WARMUP_MM):
            nc.tensor.matmul(ps0[:, 0:256], lhsT=dmy[:, :, 0:128],
                             rhs=dmy[:, :, :], start=True, stop=True,
                             perf_mode=DR)

        acc_touched = [False] * NACC
        for j, tiles_ in enumerate(CHUNKS):
            ps = ps0 if j == 0 else psum_pool.tile([128, 2048], f32, tag="ps",
                                                   name=f"ps{j}")
            # chunk0 h-outer: its h0 matmuls only wait on the xw-h0 DMA.
            ghs = ([(g, h) for h in range(2) for g in range(NG)] if j == 0
                   else [(g, h) for g in range(NG) for h in range(2)])
            for ti, t in enumerate(tiles_):
                for g, h in ghs:
                    c0 = ti * 1024 + h * NTILE
                    nc.tensor.matmul(
                        ps[:, c0:c0 + NTILE],
                        lhsT=e_sb[:, t, g, :, :],
                        rhs=xw_sb[:, h, g, :, :],
                        start=(g == 0), stop=(g == NG - 1),
                        perf_mode=DR)

            t0 = tiles_[0]
            w = 1024 * len(tiles_)
            direct = len(tiles_) == 1 and t0 in ACT_DIRECT
            tmp = (accs[ACT_DIRECT[t0]] if direct
                   else tmp_pool.tile([128, 2048], bf16, tag="tmp"))
            nc.scalar.activation(
                out=tmp[:, 0:w],
                in_=ps[:, 0:w],
                func=mybir.ActivationFunctionType.Exp,
                bias=bias_sb[:, j:j + 1],
                scale=act_scale,
            )
            if direct:
                nc.gpsimd.dma_start(out=out_d[ACT_DIRECT[t0], :, :],
                                    in_=tmp[:, :])
                continue
            for ti, t in enumerate(tiles_):
                a = accs[_acc_of_tile(t)]
                sl = tmp[:, ti * 1024:(ti + 1) * 1024]
                if not acc_touched[_acc_of_tile(t)]:
                    acc_touched[_acc_of_tile(t)] = True
                    nc.vector.tensor_scalar_mul(a[:, :], sl, 1.0)
                else:
                    nc.vector.tensor_tensor(out=a[:, :], in0=a[:, :], in1=sl,
                                            op=ADD)
            for ai, jlast in ACC_LAST_CHUNK.items():
                if j == jlast:
                    nc.gpsimd.dma_start(out=out_d[ai, :, :], in_=accs[ai][:, :])

    nc.compile()
    return nc


def _pack(labels, e2w, beta):
    """Pack exemplars into per-core [128 partition, NT slot] grids.

    Each partition holds <=NT exemplars of ONE class, consecutive in
    e2w-sorted order (so Act-chunk pair-mean biases are accurate).
    Returns per-core (grid_idx [128,NT] int64 (-1 pad), bias [128,9] f32,
    pad_const [128] f64, cls_of_part [128] int64 (-1 unused)).
    """
    chunks = []  # (class, np.array of exemplar ids, e2w-sorted)
    for c in range(C):
        idx = np.where(labels == c)[0]
        idx = idx[np.argsort(e2w[idx], kind="stable")]
        for s in range(0, len(idx), NT):
            chunks.append((c, idx[s:s + NT]))

    per_core = [[] for _ in range(NCORES)]
    for k, ch in enumerate(chunks):
        per_core[k % NCORES].append(ch)
    assert max(len(p) for p in per_core) <= 128, \
        f"chunk packing overflow: {[len(p) for p in per_core]}"

    out = []
    bf16 = None
    for cid in range(NCORES):
        grid = np.full((128, NT), -1, dtype=np.int64)
        cls = np.full(128, -1, dtype=np.int64)
        for p, (c, ids) in enumerate(per_core[cid]):
            grid[p, :len(ids)] = ids
            cls[p] = c
        # biases per chunk (pair-mean of -beta*e2w over real slots)
        bias = np.full((128, len(CHUNKS)), PAD_BIAS, dtype=np.float64)
        npad = np.zeros((128, len(CHUNKS)), dtype=np.int64)
        e2w_g = np.where(grid >= 0, e2w[grid.clip(0)], np.nan)
        for j, tiles_ in enumerate(CHUNKS):
            vals = e2w_g[:, list(tiles_)]
            cnt = np.sum(~np.isnan(vals), axis=1)
            m = cnt > 0
            bias[m, j] = -beta * np.nanmean(vals[m], axis=1)
            npad[:, j] = np.where(m, len(tiles_) - cnt, 0)
        bias_f32 = bias.astype(np.float32)
        # padding slots in half-real chunks contribute exp(bias) per pad
        # (their psum column is exactly 0); subtract on host. Round
        # through bf16 to match the device's Act output dtype.
        import concourse.mybir as mybir
        if bf16 is None:
            bf16 = _np_dt(mybir, "bfloat16")
        pad_term = np.exp(bias_f32.astype(np.float64))
        pad_term = pad_term.astype(bf16).astype(np.float64)
        pad_const = np.sum(npad * pad_term, axis=1)
        out.append((grid, bias_f32, pad_const, cls))
    return out


def _prepare(x, ex_feats, ex_labels, w_unconstrained, gamma_unconstrained,
             beta_unconstrained):
    from concourse import mybir

    x = np.asarray(x, dtype=np.float64)
    e = np.asarray(ex_feats, dtype=np.float64)
    labels = np.asarray(ex_labels).astype(np.int64)
    wu = np.asarray(w_unconstrained, dtype=np.float64)

    beta = float(np.log1p(np.exp(np.float64(beta_unconstrained)))) + EPS
    gamma = float(np.log1p(np.exp(np.float64(gamma_unconstrained)))) + EPS
    wexp = np.exp(wu - wu.max())
    w = wexp / wexp.sum() + EPS

    fp8 = _np_dt(mybir, "float8e4")

    x2w = (x * x) @ w                                 # (B,)
    e2w = (e * e) @ w                                 # (N,)
    e8 = np.ascontiguousarray(e.astype(fp8))          # (N, D)

    # xw_t[r, h, g, s, b'] = S * x[h*512+b', g*256+s*128+r] * w[...]
    xw = (S_SCALE * (x * w[None, :])).astype(np.float32)
    xw_t = np.ascontiguousarray(
        xw.reshape(2, NTILE, NG, 2, 128).transpose(4, 0, 2, 3, 1)).astype(fp8)

    packs = _pack(labels, e2w, beta)
    per_core = []
    for cid in range(NCORES):
        grid, bias_f32, pad_const, cls = packs[cid]
        gf8 = e8[grid.clip(0)]                        # (128, NT, D)
        gf8[grid < 0] = fp8(0.0)
        # e_t[r, t, g, s, p] = gf8[p, t, g*256+s*128+r]
        e_t = np.ascontiguousarray(
            gf8.reshape(128, NT, NG, 2, 128).transpose(4, 1, 2, 3, 0))
        per_core.append({"e_t": e_t, "xw_t": xw_t, "bias": bias_f32})
    return per_core, packs, x2w, beta, gamma


def kernel(x, ex_feats, ex_labels, w_unconstrained, gamma_unconstrained,
           beta_unconstrained, _want_results=False, **run_kwargs):
    from concourse.bass_utils import run_bass_kernel_spmd

    per_core, packs, x2w, beta, gamma = _prepare(
        x, ex_feats, ex_labels, w_unconstrained, gamma_unconstrained,
        beta_unconstrained)

    act_scale = float(2.0 * beta / S_SCALE)
    key = round(act_scale, 12)
    if key not in _prog_cache:
        _prog_cache[key] = _build_program(act_scale)
    nc = _prog_cache[key]

    res = run_bass_kernel_spmd(nc, per_core, list(range(NCORES)), **run_kwargs)

    class_sum = np.zeros((B, C), dtype=np.float64)
    for cid in range(NCORES):
        acc = np.asarray(res.results[cid]["acc"]).astype(np.float64)
        part = acc.sum(axis=0)                        # (128, B)
        grid, bias_f32, pad_const, cls = packs[cid]
        part -= pad_const[:, None]
        for c in range(C):
            m = cls == c
            if m.any():
                class_sum[:, c] += part[m].sum(axis=0)

    class_sum *= np.exp(-beta * x2w)[:, None]
    logits = (gamma * np.log(class_sum + EPS)).astype(np.float32)
    if _want_results:
        return logits, res
    return logits
